# revision 6
# baseline (speedup 1.0000x reference)
"""Causal self-attention (B=4, S=2048, C=1024, H=16) on 8 trn2 NeuronCores.

Sharding: core = (batch b in 0..3) x (head-group hg in 0..1), 8 heads/core.
Megatron-style TP: w_qkv column-sharded, w_proj row-sharded per head-group;
each core computes a partial projection output for its batch, host sums the
two partials per batch (collective-free).

Structure (phase-interleaved so qkv matmuls fill attention's ACT-exp gaps):
  A(0,1): v = x W_v (+leading ones col) and qT,kT for pairs 0,1, streamed
          per s-block from shared x tiles (x/wqkv in bf16; q/k stored bf16)
          v stored twice: fp8e4 (80B-padded rows for DoubleRow ldweights)
          for j>=1 blocks + bf16 for the j=0 block
  B(0):   per sq-block j, per sk-chunk-pair g:
            scoresT = kT.T@qT (bf16 K=64, head pair row-tiled);
            wT = exp(.125*s) (ACT, psum->sbuf) -> fp8e4 (j>=1) / bf16 (j=0);
            causal masking only on the 128-wide diagonal bands (DVE 0/1
            multiply after exp; below-band garbage of the second slot of a
            diagonal pair is zeroed by the mask so fp8 DoubleRow pair
            matmuls can stream both slots);
            outT[65,sq] += v.T @ wT -- fp8 DoubleRow pair matmuls (2 sk
            chunks per instruction, 0.5 cyc/row) for j>=1, bf16 per-chunk
            for j=0 (position-0 rows return v exactly; fp8 v would inject
            ~6% of |v| there). ROW 0 = softmax denom -> lane-aligned
            reciprocal straight from PSUM + partition_broadcast
  A(2,3) then B(1..3) round-robin by sq-block
  C: out_part = attn_outT.T @ w_proj_rows + b_proj (fp32r)
"""
import numpy as np
import ml_dtypes

import concourse.bass as bass
import concourse.mybir as mybir
import concourse.tile as tile
from concourse import bacc
from concourse.bass_utils import run_bass_kernel_spmd

P = 128
B, S, C, H, D = 4, 2048, 1024, 16, 64
HG = 8                 # heads per core
HD = HG * D            # 512 head dims per core
KC = C // P            # 8 contraction chunks for qkv
SB = 4                 # s blocks of 512
SQ = S // SB           # 512
VP = 80                # fp8 v row pad: pair-dim stride must be %16==0

BF16 = ml_dtypes.bfloat16
F8 = ml_dtypes.float8_e4m3

_RUNNER = None


def _build_program():
    nc = bacc.Bacc("TRN2", target_bir_lowering=False)
    f32 = mybir.dt.float32
    f32r = mybir.dt.float32r
    bf16 = mybir.dt.bfloat16
    f8 = mybir.dt.float8e4
    DR = mybir.MatmulPerfMode.DoubleRow

    xT = nc.dram_tensor("xT", [C, S], bf16, kind="ExternalInput")
    wqkv = nc.dram_tensor("wqkv", [C, 3 * HD], bf16, kind="ExternalInput")
    bqk = nc.dram_tensor("bqk", [2 * HD], f32, kind="ExternalInput")
    bv = nc.dram_tensor("bv", [HD], f32, kind="ExternalInput")
    wproj = nc.dram_tensor("wproj", [HD, C], f32r, kind="ExternalInput")
    bproj = nc.dram_tensor("bproj", [C], f32, kind="ExternalInput")
    mtrib = nc.dram_tensor("mtrib", [P, P], bf16, kind="ExternalInput")
    mtri8 = nc.dram_tensor("mtri8", [P, P], f8, kind="ExternalInput")
    mzt8 = nc.dram_tensor("mzt8", [P, 2 * P], f8, kind="ExternalInput")
    vones8 = nc.dram_tensor("vones8", [P, HG], f8, kind="ExternalInput")
    vonesb = nc.dram_tensor("vonesb", [P, HG], bf16, kind="ExternalInput")
    out = nc.dram_tensor("out_part", [S, C], f32, kind="ExternalOutput")

    xT_r = xT[:].rearrange("(kc p) s -> kc p s", p=P)
    wqk_r = wqkv[:, 0:2 * HD].rearrange("(kc p) n -> kc p n", p=P)
    wv_r = wqkv[:, 2 * HD:3 * HD].rearrange("(kc p) n -> kc p n", p=P)

    with tile.TileContext(nc) as tc:
        with (
            tc.tile_pool(name="persist", bufs=1) as pp,
            tc.tile_pool(name="small", bufs=1) as sp,
        ):
            qkT = [
                pp.tile([P, S], bf16, tag=f"qkT{i}", name=f"qkT{i}")
                for i in range(8)
            ]
            # fp8 v for j>=1 DoubleRow pairs; bf16 v for the j=0 block
            v8 = pp.tile([P, S // P, HG, VP], f8, tag="v8")
            vbb = pp.tile([P, 4, HG, D + 1], bf16, tag="vbb")

            bqk_sb = sp.tile([P, 2 * HD // P], f32, tag="bqk")
            nc.sync.dma_start(bqk_sb[:], bqk[:].rearrange("(blk p) -> p blk", p=P))
            bv_bc = sp.tile([P, HD], f32, tag="bv_bc")
            nc.sync.dma_start(bv_bc[:], bv[:].unsqueeze(0).to_broadcast((P, HD)))
            bp_bc = sp.tile([P, C], f32, tag="bp_bc")
            # ones column FIRST in v: denominator lands on psum partition 0
            ones8 = sp.tile([P, HG], f8, tag="ones8")
            nc.sync.dma_start(ones8[:], vones8[:])
            onesb = sp.tile([P, HG], bf16, tag="onesb")
            nc.sync.dma_start(onesb[:], vonesb[:])
            for st in range(S // P):
                nc.vector.tensor_copy(v8[:, st, :, 0], ones8[:])
            for st in range(4):
                nc.vector.tensor_copy(vbb[:, st, :, 0], onesb[:])

            # ---- interleaved A (qkv, v merged into first x sweep) + B ----
            with tc.tile_pool(name="persistBC", bufs=1) as pbc:
                aT = pbc.tile([P, HD // P, S], f32r, tag="attn_outT")
                neg3 = pbc.tile([P, 1], f32, tag="neg3")
                nc.vector.memset(neg3[:], -3.0)
                trib = pbc.tile([P, P], bf16, tag="trib")
                tri8 = pbc.tile([P, P], f8, tag="tri8")
                mz8 = pbc.tile([P, 2 * P], f8, tag="mz8")

                from contextlib import ExitStack
                stack = ExitStack()
                with stack:
                    xp = stack.enter_context(tc.tile_pool(name="xpool", bufs=2))
                    wp = stack.enter_context(tc.tile_pool(name="wpool", bufs=1))

                    psA1 = stack.enter_context(
                        tc.tile_pool(name="psA1", bufs=2, space="PSUM")
                    )

                    def open_b_pools():
                        wtp = stack.enter_context(
                            tc.tile_pool(name="wtpool", bufs=4)
                        )
                        psS = stack.enter_context(
                            tc.tile_pool(name="psS", bufs=1, space="PSUM")
                        )
                        psO = stack.enter_context(
                            tc.tile_pool(name="psO", bufs=1, space="PSUM")
                        )
                        rcp = stack.enter_context(
                            tc.tile_pool(name="rcpool", bufs=2)
                        )
                        return wtp, psS, psO, rcp

                    vstack = ExitStack()
                    wvp = vstack.enter_context(
                        tc.tile_pool(name="wvpool", bufs=1)
                    )
                    wv_k = [
                        wvp.tile([P, HD], bf16, tag=f"wv{kc}", name=f"wv{kc}")
                        for kc in range(KC)
                    ]

                    def load_wv():
                        for kc in range(KC):
                            nc.sync.dma_start(wv_k[kc][:], wv_r[kc])

                    def emit_a1(pairs, with_v=False):
                        ocs = [hp for hp in pairs] + [4 + hp for hp in pairs]
                        wqk_t = {}

                        def load_wqk():
                            for i, oc in enumerate(ocs):
                                for kc in range(KC):
                                    wt_ = wp.tile([P, P], bf16,
                                                  tag=f"wqk{i}_{kc}",
                                                  name=f"wqk_{oc}_{kc}")
                                    nc.sync.dma_start(
                                        wt_[:],
                                        wqk_r[kc, :, oc * P:(oc + 1) * P],
                                    )
                                    wqk_t[(oc, kc)] = wt_

                        def qk_mm(oc, xk, sb):
                            ps = psA1.tile([P, SQ], f32, tag="psA1")
                            for kc in range(KC):
                                nc.tensor.matmul(
                                    ps[:],
                                    wqk_t[(oc, kc)][:],
                                    xk[kc][:],
                                    start=(kc == 0),
                                    stop=(kc == KC - 1),
                                )
                            nc.vector.tensor_scalar_add(
                                qkT[oc][:, sb * SQ:(sb + 1) * SQ],
                                ps[:],
                                bqk_sb[:, oc:oc + 1],
                            )

                        def v_mm(sb, xk):
                            for stl in range(SQ // P):
                                st = sb * (SQ // P) + stl
                                ps = psA1.tile([P, HD], f32, tag="psA1",
                                               name=f"psV_{sb}_{stl}")
                                for kc in range(KC):
                                    nc.tensor.matmul(
                                        ps[:],
                                        xk[kc][:, stl * P:(stl + 1) * P],
                                        wv_k[kc][:],
                                        start=(kc == 0),
                                        stop=(kc == KC - 1),
                                    )
                                nc.vector.tensor_add(
                                    out=v8[:, st, :, 1:D + 1],
                                    in0=ps[:].rearrange(
                                        "p (h d) -> p h d", h=HG),
                                    in1=bv_bc[:].rearrange(
                                        "p (h d) -> p h d", h=HG),
                                )
                                if st < 4:
                                    nc.vector.tensor_add(
                                        out=vbb[:, st, :, 1:D + 1],
                                        in0=ps[:].rearrange(
                                            "p (h d) -> p h d", h=HG),
                                        in1=bv_bc[:].rearrange(
                                            "p (h d) -> p h d", h=HG),
                                    )

                        if not with_v:
                            load_wqk()
                        for sb in range(SB):
                            if with_v and sb == 0:
                                load_wqk()
                            xk = [
                                xp.tile([P, SQ], bf16, tag=f"x{kc}",
                                        name=f"x_{pairs[0]}_{sb}_{kc}")
                                for kc in range(KC)
                            ]
                            for kc in range(KC):
                                nc.sync.dma_start(
                                    xk[kc][:], xT_r[kc, :, sb * SQ:(sb + 1) * SQ]
                                )
                            if with_v and sb == 0:
                                # qk first: PE starts on the first x tile;
                                # wv weights stream behind the startup path
                                for oc in ocs:
                                    qk_mm(oc, xk, sb)
                                load_wv()
                                v_mm(sb, xk)
                            elif with_v:
                                v_mm(sb, xk)
                                for oc in ocs:
                                    qk_mm(oc, xk, sb)
                            else:
                                for oc in ocs:
                                    qk_mm(oc, xk, sb)

                    bp = {}

                    def emit_b_unit(hp, j):
                        psO = bp["psO"]
                        qT_blk = qkT[hp]
                        kT_blk = qkT[4 + hp]
                        ngrp = 2 * (j + 1)
                        sq = slice(j * SQ, (j + 1) * SQ)
                        po = [
                            psO.tile([D + 1, SQ], f32, tag=f"psO{h}",
                                     name=f"psO_{hp}_{j}_{h}")
                            for h in range(2)
                        ]
                        if j == 0:
                            # bf16 path, per-chunk AV, per-slot trim
                            for g in range(2):
                                for h in range(2):
                                    p0 = h * D
                                    habs = hp * 2 + h
                                    pss = psS.tile([P, 2, SQ], f32,
                                                   tag=f"psS{h}",
                                                   name=f"psS_{hp}_{j}_{g}_{h}")
                                    wT = wtp.tile([P, 2, SQ], bf16, tag="wTb")
                                    for u in range(2):
                                        t = 2 * g + u
                                        o = t * P
                                        nc.tensor.matmul(
                                            pss[:, u, o:SQ],
                                            kT_blk[p0:p0 + D,
                                                   t * P:(t + 1) * P],
                                            qT_blk[p0:p0 + D,
                                                   j * SQ + o:(j + 1) * SQ],
                                            start=True,
                                            stop=True,
                                        )
                                        nc.scalar.activation(
                                            wT[:, u, o:SQ], pss[:, u, o:SQ],
                                            mybir.ActivationFunctionType.Exp,
                                            scale=0.125,
                                        )
                                        nc.vector.tensor_mul(
                                            out=wT[:, u, o:o + P],
                                            in0=wT[:, u, o:o + P],
                                            in1=trib[:],
                                        )
                                    for u in range(2):
                                        t = 2 * g + u
                                        o = t * P
                                        nc.tensor.matmul(
                                            po[h][:, o:SQ],
                                            vbb[:, t, habs, :],
                                            wT[:, u, o:SQ],
                                            start=(t == 0),
                                            stop=(t == 3),
                                        )
                        else:
                            # fp8 DoubleRow path: pair matmuls, 2 chunks each
                            for g in range(ngrp):
                                diag2 = g == ngrp - 1
                                o = 2 * P if diag2 else 0
                                for h in range(2):
                                    p0 = h * D
                                    habs = hp * 2 + h
                                    pss = psS.tile([P, 2, SQ], f32,
                                                   tag=f"psS{h}",
                                                   name=f"psS_{hp}_{j}_{g}_{h}")
                                    wT = wtp.tile([P, 2, SQ], f8, tag="wT8")
                                    for u in range(2):
                                        t = 2 * g + u
                                        nc.tensor.matmul(
                                            pss[:, u, o:SQ],
                                            kT_blk[p0:p0 + D,
                                                   t * P:(t + 1) * P],
                                            qT_blk[p0:p0 + D,
                                                   j * SQ + o:(j + 1) * SQ],
                                            start=True,
                                            stop=True,
                                        )
                                    # shift by -3 (softmax-invariant per row:
                                    # j>=1 rows use only fp8 chunks): keeps
                                    # exp <= e^2.5 = 12.2, under fp8e4's 240
                                    # max; sub-2^-9 weights flush to 0
                                    # (<0.1% of any denominator)
                                    nc.scalar.activation(
                                        wT[:, :, o:SQ], pss[:, :, o:SQ],
                                        mybir.ActivationFunctionType.Exp,
                                        scale=0.125,
                                        bias=neg3[:],
                                    )
                                    if g >= ngrp - 2:
                                        # causal band masks (post-exp):
                                        # slot0 tri at its band; slot1 zero
                                        # below-band + tri (mz8) so the pair
                                        # matmul can stream both slots
                                        nc.vector.tensor_mul(
                                            out=wT[:, 0, o:o + P],
                                            in0=wT[:, 0, o:o + P],
                                            in1=tri8[:],
                                        )
                                        nc.vector.tensor_mul(
                                            out=wT[:, 1, o:o + 2 * P],
                                            in0=wT[:, 1, o:o + 2 * P],
                                            in1=mz8[:],
                                        )
                                    nc.tensor.matmul(
                                        po[h][:, o:SQ],
                                        v8[:, 2 * g:2 * g + 2, habs, 0:D + 1],
                                        wT[:, :, o:SQ],
                                        start=(g == 0),
                                        stop=(g == ngrp - 1),
                                        perf_mode=DR,
                                    )
                        for h in range(2):
                            # denom on psum partition 0 (ones col first):
                            # direct lane-aligned reciprocal from PSUM
                            src = po[h]
                            rc = rcp.tile([1, SQ], f32, tag="rc")
                            nc.vector.reciprocal(rc[:], src[0:1, :])
                            rcb = rcp.tile([D + 1, SQ], f32, tag="rcb")
                            nc.gpsimd.partition_broadcast(rcb[:], rc[:])
                            # engines need 32-aligned partition bases:
                            # multiply all 65 rows (row 0 harmless), DMA
                            # extracts rows 1..64
                            nt = rcp.tile([D + 1, SQ], f32r, tag="nt")
                            nc.vector.tensor_mul(
                                out=nt[:], in0=src[:], in1=rcb[:],
                            )
                            nc.sync.dma_start(
                                aT[h * D:(h + 1) * D, hp, sq],
                                nt[1:D + 1, :],
                            )

                    emit_a1((0, 1), with_v=True)
                    # non-critical loads emitted after the startup-critical
                    # x/wqk/wv stream
                    nc.sync.dma_start(trib[:], mtrib[:])
                    nc.sync.dma_start(tri8[:], mtri8[:])
                    nc.sync.dma_start(mz8[:], mzt8[:])
                    nc.sync.dma_start(
                        bp_bc[:], bproj[:].unsqueeze(0).to_broadcast((P, C))
                    )
                    vstack.close()  # free wv weights before B pools open
                    wtp, psS, psO, rcp = open_b_pools()
                    bp["psO"] = psO
                    for j in range(SB):
                        for pair in (0, 1):
                            emit_b_unit(pair, j)
                    emit_a1((2, 3))
                    for j in range(SB):
                        for pair in (2, 3):
                            emit_b_unit(pair, j)

                # ---- Phase C: projection ----
                with (
                    tc.tile_pool(name="wppool", bufs=1) as wpp,
                    tc.tile_pool(name="opool", bufs=4) as op,
                    tc.tile_pool(name="psC", bufs=2, space="PSUM") as psC,
                ):
                    wp_sb = wpp.tile([P, HD // P, C], f32r, tag="wp_sb")
                    nc.sync.dma_start(
                        wp_sb[:], wproj[:].rearrange("(hp p) n -> p hp n", p=P)
                    )
                    for st in range(S // P):
                        for ocb in range(2):
                            nsl = slice(ocb * SQ, (ocb + 1) * SQ)
                            ps = psC.tile([P, SQ], f32, tag="psC")
                            for hp in range(HD // P):
                                nc.tensor.matmul(
                                    ps[:],
                                    aT[:, hp, st * P:(st + 1) * P],
                                    wp_sb[:, hp, nsl],
                                    start=(hp == 0),
                                    stop=(hp == HD // P - 1),
                                )
                            ot = op.tile([P, SQ], f32, tag="ot")
                            nc.vector.tensor_add(
                                out=ot[:], in0=ps[:], in1=bp_bc[:, nsl]
                            )
                            nc.sync.dma_start(out[st * P:(st + 1) * P, nsl], ot[:])

    nc.compile()
    return nc


def _make_masks():
    # tri[p, c] = 1 iff c >= p  (within a 128-wide diagonal band)
    p = np.arange(P)[:, None]
    c = np.arange(P)[None, :]
    tri = (c >= p).astype(np.float32)
    # mzt: second slot of a diagonal pair: zeros below-band, then tri
    mzt = np.concatenate([np.zeros((P, P), np.float32), tri], axis=1)
    return tri, mzt


def _shard_inputs(x, w_qkv, b_qkv, w_proj, b_proj):
    tri, mzt = _make_masks()
    x = np.asarray(x, np.float32)
    w_qkv = np.asarray(w_qkv, np.float32)
    b_qkv = np.asarray(b_qkv, np.float32)
    w_proj = np.asarray(w_proj, np.float32)
    b_proj = np.asarray(b_proj, np.float32)
    zeros_c = np.zeros((C,), np.float32)
    in_maps = []
    for core in range(8):
        b, hg = core // 2, core % 2
        cs = slice(hg * HD, (hg + 1) * HD)
        wq = w_qkv[:, 0:C][:, cs]
        wk = w_qkv[:, C:2 * C][:, cs]
        wv = w_qkv[:, 2 * C:3 * C][:, cs]
        bq = b_qkv[0:C][cs]
        bk = b_qkv[C:2 * C][cs]
        bvv = b_qkv[2 * C:3 * C][cs]
        in_maps.append({
            "xT": np.ascontiguousarray(x[b].T).astype(BF16),
            "wqkv": np.ascontiguousarray(
                np.concatenate([wq, wk, wv], axis=1)).astype(BF16),
            "bqk": np.ascontiguousarray(np.concatenate([bq, bk])),
            "bv": np.ascontiguousarray(bvv),
            "wproj": np.ascontiguousarray(w_proj[cs, :]),
            "bproj": b_proj if hg == 0 else zeros_c,
            "mtrib": tri.astype(BF16),
            "mtri8": tri.astype(F8),
            "mzt8": mzt.astype(F8),
            "vones8": np.ones((P, HG), np.float32).astype(F8),
            "vonesb": np.ones((P, HG), np.float32).astype(BF16),
        })
    return in_maps


def get_program():
    global _RUNNER
    if _RUNNER is None:
        _RUNNER = _build_program()
    return _RUNNER


def kernel(x, w_qkv, b_qkv, w_proj, b_proj):
    nc = get_program()
    in_maps = _shard_inputs(x, w_qkv, b_qkv, w_proj, b_proj)
    res = run_bass_kernel_spmd(nc, in_maps, list(range(8)))
    out = np.empty((B, S, C), np.float32)
    for b in range(B):
        out[b] = res.results[2 * b]["out_part"] + res.results[2 * b + 1]["out_part"]
    return out


# revision 14
# speedup vs baseline: 1.0176x; 1.0176x over previous
"""Causal self-attention (B=4, S=2048, C=1024, H=16) on 8 trn2 NeuronCores.

Sharding: core = (batch b in 0..3) x (head-group hg in 0..1), 8 heads/core.
Megatron-style TP: w_qkv column-sharded, w_proj row-sharded per head-group;
each core computes a partial projection output for its batch, host sums the
two partials per batch (collective-free).

Structure (phase-interleaved so qkv matmuls fill attention's ACT-exp gaps):
  A(0,1): v = x W_v (+leading ones col) and qT,kT for pairs 0,1, streamed
          per s-block from shared x tiles (x/wqkv in bf16; q/k stored bf16)
          v stored twice: fp8e4 (80B-padded rows for DoubleRow ldweights)
          for j>=1 blocks + bf16 for the j=0 block
  B(0):   per sq-block j, per sk-chunk-pair g:
            scoresT = kT.T@qT (bf16 K=64, head pair row-tiled);
            wT = exp(.125*s) (ACT, psum->sbuf) -> fp8e4 (j>=1) / bf16 (j=0);
            causal masking only on the 128-wide diagonal bands (DVE 0/1
            multiply after exp; below-band garbage of the second slot of a
            diagonal pair is zeroed by the mask so fp8 DoubleRow pair
            matmuls can stream both slots);
            outT[65,sq] += v.T @ wT -- fp8 DoubleRow pair matmuls (2 sk
            chunks per instruction, 0.5 cyc/row) for j>=1, bf16 per-chunk
            for j=0 (position-0 rows return v exactly; fp8 v would inject
            ~6% of |v| there). ROW 0 = softmax denom -> lane-aligned
            reciprocal straight from PSUM + partition_broadcast
  A(2,3) then B(1..3) round-robin by sq-block
  C: out_part = attn_outT.T @ w_proj_rows + b_proj (fp32r)
"""
import numpy as np
import ml_dtypes

import concourse.bass as bass
import concourse.mybir as mybir
import concourse.tile as tile
from concourse import bacc
from concourse.bass_utils import run_bass_kernel_spmd

P = 128
B, S, C, H, D = 4, 2048, 1024, 16, 64
HG = 8                 # heads per core
HD = HG * D            # 512 head dims per core
KC = C // P            # 8 contraction chunks for qkv
SB = 4                 # s blocks of 512
SQ = S // SB           # 512
VP = 80                # fp8 v row pad: pair-dim stride must be %16==0

BF16 = ml_dtypes.bfloat16
F8 = ml_dtypes.float8_e4m3

_RUNNER = None


def _build_program():
    nc = bacc.Bacc("TRN2", target_bir_lowering=False)
    f32 = mybir.dt.float32
    f32r = mybir.dt.float32r
    bf16 = mybir.dt.bfloat16
    f8 = mybir.dt.float8e4
    DR = mybir.MatmulPerfMode.DoubleRow

    xT = nc.dram_tensor("xT", [C, S], bf16, kind="ExternalInput")
    wqkv = nc.dram_tensor("wqkv", [C, 3 * HD], bf16, kind="ExternalInput")
    bqk = nc.dram_tensor("bqk", [2 * HD], f32, kind="ExternalInput")
    bv = nc.dram_tensor("bv", [HD], f32, kind="ExternalInput")
    wproj = nc.dram_tensor("wproj", [HD, C], f32r, kind="ExternalInput")
    bproj = nc.dram_tensor("bproj", [C], f32, kind="ExternalInput")
    mtrib = nc.dram_tensor("mtrib", [P, P], bf16, kind="ExternalInput")
    mtri8 = nc.dram_tensor("mtri8", [P, P], f8, kind="ExternalInput")
    mzt8 = nc.dram_tensor("mzt8", [P, 2 * P], f8, kind="ExternalInput")
    vones8 = nc.dram_tensor("vones8", [P, HG], f8, kind="ExternalInput")
    vonesb = nc.dram_tensor("vonesb", [P, HG], bf16, kind="ExternalInput")
    out = nc.dram_tensor("out_part", [S, C], f32, kind="ExternalOutput")

    xT_r = xT[:].rearrange("(kc p) s -> kc p s", p=P)
    wqk_r = wqkv[:, 0:2 * HD].rearrange("(kc p) n -> kc p n", p=P)
    wv_r = wqkv[:, 2 * HD:3 * HD].rearrange("(kc p) n -> kc p n", p=P)

    with tile.TileContext(nc) as tc:
        with (
            tc.tile_pool(name="persist", bufs=1) as pp,
            tc.tile_pool(name="small", bufs=1) as sp,
        ):
            qkT = [
                pp.tile([P, S], bf16, tag=f"qkT{i}", name=f"qkT{i}")
                for i in range(8)
            ]
            # fp8 v for j>=1 DoubleRow pairs; bf16 v for the j=0 block
            v8 = pp.tile([P, S // P, HG, VP], f8, tag="v8")
            vbb = pp.tile([P, 4, HG, D + 1], bf16, tag="vbb")

            bqk_sb = sp.tile([P, 2 * HD // P], f32, tag="bqk")
            nc.sync.dma_start(bqk_sb[:], bqk[:].rearrange("(blk p) -> p blk", p=P))
            bv_bc = sp.tile([P, HD], f32, tag="bv_bc")
            nc.sync.dma_start(bv_bc[:], bv[:].unsqueeze(0).to_broadcast((P, HD)))
            bp_bc = sp.tile([P, C], f32, tag="bp_bc")
            # ones column FIRST in v: denominator lands on psum partition 0
            ones8 = sp.tile([P, HG], f8, tag="ones8")
            nc.sync.dma_start(ones8[:], vones8[:])
            onesb = sp.tile([P, HG], bf16, tag="onesb")
            nc.sync.dma_start(onesb[:], vonesb[:])
            for st in range(S // P):
                nc.vector.tensor_copy(v8[:, st, :, 0], ones8[:])
            for st in range(4):
                nc.vector.tensor_copy(vbb[:, st, :, 0], onesb[:])

            # ---- interleaved A (qkv, v merged into first x sweep) + B ----
            with tc.tile_pool(name="persistBC", bufs=1) as pbc:
                aT = pbc.tile([P, HD // P, S], f32r, tag="attn_outT")
                neg3 = pbc.tile([P, 1], f32, tag="neg3")
                nc.vector.memset(neg3[:], -3.0)
                trib = pbc.tile([P, P], bf16, tag="trib")
                tri8 = pbc.tile([P, P], f8, tag="tri8")
                mz8 = pbc.tile([P, 2 * P], f8, tag="mz8")

                from contextlib import ExitStack
                stack = ExitStack()
                with stack:
                    # B pools open first (outermost) so the A-phase stack
                    # below can close mid-kernel in proper LIFO order
                    wtp = stack.enter_context(
                        tc.tile_pool(name="wtpool", bufs=4)
                    )
                    psS = stack.enter_context(
                        tc.tile_pool(name="psS", bufs=1, space="PSUM")
                    )
                    psO = stack.enter_context(
                        tc.tile_pool(name="psO", bufs=1, space="PSUM")
                    )
                    rcp = stack.enter_context(
                        tc.tile_pool(name="rcpool", bufs=2)
                    )
                    # A-phase pools in their own stack: psA1's 2 PSUM banks
                    # are released before the projection's psC pool opens
                    astack = ExitStack()
                    xp = astack.enter_context(tc.tile_pool(name="xpool", bufs=2))
                    wp = astack.enter_context(tc.tile_pool(name="wpool", bufs=1))
                    psA1 = astack.enter_context(
                        tc.tile_pool(name="psA1", bufs=2, space="PSUM")
                    )

                    vstack = ExitStack()
                    wvp = vstack.enter_context(
                        tc.tile_pool(name="wvpool", bufs=1)
                    )
                    wv_t = wvp.tile([P, KC, HD], bf16, tag="wv")

                    def load_wv():
                        nc.sync.dma_start(
                            wv_t[:],
                            wv_r.rearrange("kc p n -> p kc n"),
                        )

                    def emit_a1(pairs, with_v=False):
                        ocs = [hp for hp in pairs] + [4 + hp for hp in pairs]
                        wqk_t = {}

                        def load_wqk():
                            for i, oc in enumerate(ocs):
                                wt_ = wp.tile([P, KC, P], bf16,
                                              tag=f"wqk{i}",
                                              name=f"wqk_{oc}")
                                nc.sync.dma_start(
                                    wt_[:],
                                    wqk_r[:, :, oc * P:(oc + 1) * P]
                                    .rearrange("kc p n -> p kc n"),
                                )
                                wqk_t[oc] = wt_

                        def qk_mm(oc, xk, sb):
                            ps = psA1.tile([P, SQ], f32, tag="psA1")
                            for kc in range(KC):
                                nc.tensor.matmul(
                                    ps[:],
                                    wqk_t[oc][:, kc, :],
                                    xk[:, kc, :],
                                    start=(kc == 0),
                                    stop=(kc == KC - 1),
                                )
                            nc.vector.tensor_scalar_add(
                                qkT[oc][:, sb * SQ:(sb + 1) * SQ],
                                ps[:],
                                bqk_sb[:, oc:oc + 1],
                            )

                        def v_mm(sb, xk):
                            for stl in range(SQ // P):
                                st = sb * (SQ // P) + stl
                                ps = psA1.tile([P, HD], f32, tag="psA1",
                                               name=f"psV_{sb}_{stl}")
                                for kc in range(KC):
                                    nc.tensor.matmul(
                                        ps[:],
                                        xk[:, kc, stl * P:(stl + 1) * P],
                                        wv_t[:, kc, :],
                                        start=(kc == 0),
                                        stop=(kc == KC - 1),
                                    )
                                nc.vector.tensor_add(
                                    out=v8[:, st, :, 1:D + 1],
                                    in0=ps[:].rearrange(
                                        "p (h d) -> p h d", h=HG),
                                    in1=bv_bc[:].rearrange(
                                        "p (h d) -> p h d", h=HG),
                                )
                                if st < 4:
                                    nc.vector.tensor_add(
                                        out=vbb[:, st, :, 1:D + 1],
                                        in0=ps[:].rearrange(
                                            "p (h d) -> p h d", h=HG),
                                        in1=bv_bc[:].rearrange(
                                            "p (h d) -> p h d", h=HG),
                                    )

                        if not with_v:
                            load_wqk()
                        for sb in range(SB):
                            if with_v and sb == 0:
                                load_wqk()
                            xk = xp.tile([P, KC, SQ], bf16, tag="x",
                                         name=f"x_{pairs[0]}_{sb}")
                            nc.sync.dma_start(
                                xk[:],
                                xT_r[:, :, sb * SQ:(sb + 1) * SQ]
                                .rearrange("kc p s -> p kc s"),
                            )
                            if with_v and sb == 0:
                                # qk first: PE starts on the first x tile;
                                # wv weights stream behind the startup path
                                for oc in ocs:
                                    qk_mm(oc, xk, sb)
                                load_wv()
                                v_mm(sb, xk)
                            elif with_v:
                                v_mm(sb, xk)
                                for oc in ocs:
                                    qk_mm(oc, xk, sb)
                            else:
                                for oc in ocs:
                                    qk_mm(oc, xk, sb)

                    def emit_a23_gen():
                        # A(2,3) as a filler generator: one qk chain per
                        # yield, emitted between B groups so the in-order PE
                        # queue has work during B's ACT-bound stretches
                        ocs = [2, 3, 6, 7]
                        wqk_t = {}
                        for i, oc in enumerate(ocs):
                            wt_ = wp.tile([P, KC, P], bf16,
                                          tag=f"wqk{i}",
                                          name=f"wqk23_{oc}")
                            nc.sync.dma_start(
                                wt_[:],
                                wqk_r[:, :, oc * P:(oc + 1) * P]
                                .rearrange("kc p n -> p kc n"),
                            )
                            wqk_t[oc] = wt_

                        def gen():
                            for sb in range(SB):
                                xk = xp.tile([P, KC, SQ], bf16, tag="x",
                                             name=f"x23_{sb}")
                                nc.sync.dma_start(
                                    xk[:],
                                    xT_r[:, :, sb * SQ:(sb + 1) * SQ]
                                    .rearrange("kc p s -> p kc s"),
                                )
                                for oc in ocs:
                                    ps = psA1.tile([P, SQ], f32, tag="psA1")
                                    for kc in range(KC):
                                        nc.tensor.matmul(
                                            ps[:],
                                            wqk_t[oc][:, kc, :],
                                            xk[:, kc, :],
                                            start=(kc == 0),
                                            stop=(kc == KC - 1),
                                        )
                                    nc.vector.tensor_scalar_add(
                                        qkT[oc][:, sb * SQ:(sb + 1) * SQ],
                                        ps[:],
                                        bqk_sb[:, oc:oc + 1],
                                    )
                                    yield
                        return gen()

                    bp = {}

                    def emit_b_unit(hp, j, filler=None):
                        psO = bp["psO"]
                        qT_blk = qkT[hp]
                        kT_blk = qkT[4 + hp]
                        ngrp = 2 * (j + 1)
                        sq = slice(j * SQ, (j + 1) * SQ)
                        po = [
                            psO.tile([D + 1, SQ], f32, tag=f"psO{h}",
                                     name=f"psO_{hp}_{j}_{h}")
                            for h in range(2)
                        ]
                        if j == 0:
                            # bf16 path, per-chunk AV, per-slot trim
                            for g in range(2):
                                for h in range(2):
                                    p0 = h * D
                                    habs = hp * 2 + h
                                    pss = psS.tile([P, 2, SQ], f32,
                                                   tag=f"psS{h}",
                                                   name=f"psS_{hp}_{j}_{g}_{h}")
                                    wT = wtp.tile([P, 2, SQ], bf16, tag="wTb")
                                    for u in range(2):
                                        t = 2 * g + u
                                        o = t * P
                                        nc.tensor.matmul(
                                            pss[:, u, o:SQ],
                                            kT_blk[p0:p0 + D,
                                                   t * P:(t + 1) * P],
                                            qT_blk[p0:p0 + D,
                                                   j * SQ + o:(j + 1) * SQ],
                                            start=True,
                                            stop=True,
                                        )
                                        nc.scalar.activation(
                                            wT[:, u, o:SQ], pss[:, u, o:SQ],
                                            mybir.ActivationFunctionType.Exp,
                                            scale=0.125,
                                        )
                                        nc.vector.tensor_mul(
                                            out=wT[:, u, o:o + P],
                                            in0=wT[:, u, o:o + P],
                                            in1=trib[:],
                                        )
                                    for u in range(2):
                                        t = 2 * g + u
                                        o = t * P
                                        nc.tensor.matmul(
                                            po[h][:, o:SQ],
                                            vbb[:, t, habs, :],
                                            wT[:, u, o:SQ],
                                            start=(t == 0),
                                            stop=(t == 3),
                                        )
                                if filler is not None:
                                    next(filler, None)
                        else:
                            # fp8 DoubleRow path: pair matmuls, 2 chunks each
                            for g in range(ngrp):
                                diag2 = g == ngrp - 1
                                o = 2 * P if diag2 else 0
                                for h in range(2):
                                    p0 = h * D
                                    habs = hp * 2 + h
                                    pss = psS.tile([P, 2, SQ], f32,
                                                   tag=f"psS{h}",
                                                   name=f"psS_{hp}_{j}_{g}_{h}")
                                    wT = wtp.tile([P, 2, SQ], f8, tag="wT8")
                                    for u in range(2):
                                        t = 2 * g + u
                                        nc.tensor.matmul(
                                            pss[:, u, o:SQ],
                                            kT_blk[p0:p0 + D,
                                                   t * P:(t + 1) * P],
                                            qT_blk[p0:p0 + D,
                                                   j * SQ + o:(j + 1) * SQ],
                                            start=True,
                                            stop=True,
                                        )
                                    # shift by -3 (softmax-invariant per row:
                                    # j>=1 rows use only fp8 chunks): keeps
                                    # exp <= e^2.5 = 12.2, under fp8e4's 240
                                    # max; sub-2^-9 weights flush to 0
                                    # (<0.1% of any denominator)
                                    nc.scalar.activation(
                                        wT[:, :, o:SQ], pss[:, :, o:SQ],
                                        mybir.ActivationFunctionType.Exp,
                                        scale=0.125,
                                        bias=neg3[:],
                                    )
                                    if g >= ngrp - 2:
                                        # causal band masks (post-exp):
                                        # slot0 tri at its band; slot1 zero
                                        # below-band + tri (mz8) so the pair
                                        # matmul can stream both slots
                                        nc.vector.tensor_mul(
                                            out=wT[:, 0, o:o + P],
                                            in0=wT[:, 0, o:o + P],
                                            in1=tri8[:],
                                        )
                                        nc.vector.tensor_mul(
                                            out=wT[:, 1, o:o + 2 * P],
                                            in0=wT[:, 1, o:o + 2 * P],
                                            in1=mz8[:],
                                        )
                                    nc.tensor.matmul(
                                        po[h][:, o:SQ],
                                        v8[:, 2 * g:2 * g + 2, habs, 0:D + 1],
                                        wT[:, :, o:SQ],
                                        start=(g == 0),
                                        stop=(g == ngrp - 1),
                                        perf_mode=DR,
                                    )
                                if filler is not None:
                                    next(filler, None)
                        for h in range(2):
                            # denom on psum partition 0 (ones col first):
                            # direct lane-aligned reciprocal from PSUM
                            src = po[h]
                            rc = rcp.tile([1, SQ], f32, tag="rc")
                            nc.vector.reciprocal(rc[:], src[0:1, :])
                            rcb = rcp.tile([D + 1, SQ], f32, tag="rcb")
                            nc.gpsimd.partition_broadcast(rcb[:], rc[:])
                            # engines need 32-aligned partition bases:
                            # multiply all 65 rows (row 0 harmless), DMA
                            # extracts rows 1..64
                            nt = rcp.tile([D + 1, SQ], f32r, tag="nt")
                            nc.vector.tensor_mul(
                                out=nt[:], in0=src[:], in1=rcb[:],
                            )
                            nc.sync.dma_start(
                                aT[h * D:(h + 1) * D, hp, sq],
                                nt[1:D + 1, :],
                            )

                    emit_a1((0, 1), with_v=True)
                    # non-critical loads emitted after the startup-critical
                    # x/wqk/wv stream
                    nc.sync.dma_start(trib[:], mtrib[:])
                    nc.sync.dma_start(tri8[:], mtri8[:])
                    nc.sync.dma_start(mz8[:], mzt8[:])
                    nc.sync.dma_start(
                        bp_bc[:], bproj[:].unsqueeze(0).to_broadcast((P, C))
                    )
                    vstack.close()  # free wv weights before B starts
                    bp["psO"] = psO
                    # B pairs (0,1); A(2,3) qk chains woven in as PE filler
                    # once B's ACT-bound stretch deepens (j >= 2)
                    fill = None
                    for j in range(SB):
                        if j == 2:
                            fill = emit_a23_gen()
                        emit_b_unit(0, j, fill)
                        emit_b_unit(1, j, fill)
                    if fill is not None:
                        for _ in fill:
                            pass
                    astack.close()  # free psA1 banks + x/wqk sbuf for proj

                    wpp = stack.enter_context(tc.tile_pool(name="wppool", bufs=1))
                    op = stack.enter_context(tc.tile_pool(name="opool", bufs=4))
                    psC = stack.enter_context(
                        tc.tile_pool(name="psC", bufs=2, space="PSUM")
                    )
                    wp_sb = wpp.tile([P, HD // P, C], f32r, tag="wp_sb")
                    nc.sync.dma_start(
                        wp_sb[:], wproj[:].rearrange("(hp p) n -> p hp n", p=P)
                    )

                    def proj_gen(j):
                        # projection for s-block j (aT rows complete once
                        # B(*, j) is done for all pairs); one psC chain
                        # per yield — PE filler for the next B2 unit
                        for stl in range(SQ // P):
                            st = j * (SQ // P) + stl
                            ot = op.tile([P, C], f32, tag="ot")
                            for ocb in range(2):
                                nsl = slice(ocb * SQ, (ocb + 1) * SQ)
                                ps = psC.tile([P, SQ], f32, tag="psC")
                                for hc in range(HD // P):
                                    nc.tensor.matmul(
                                        ps[:],
                                        aT[:, hc, st * P:(st + 1) * P],
                                        wp_sb[:, hc, nsl],
                                        start=(hc == 0),
                                        stop=(hc == HD // P - 1),
                                    )
                                nc.vector.tensor_add(
                                    out=ot[:, nsl], in0=ps[:], in1=bp_bc[:, nsl]
                                )
                                yield
                            nc.sync.dma_start(
                                out[st * P:(st + 1) * P, :], ot[:]
                            )

                    # B pairs (2,3); block-j projection woven into block j+1
                    fill = None
                    for j in range(SB):
                        emit_b_unit(2, j, fill)
                        emit_b_unit(3, j, fill)
                        if fill is not None:
                            for _ in fill:
                                pass
                        fill = proj_gen(j)
                    for _ in fill:
                        pass

    nc.compile()
    return nc


def _make_masks():
    # tri[p, c] = 1 iff c >= p  (within a 128-wide diagonal band)
    p = np.arange(P)[:, None]
    c = np.arange(P)[None, :]
    tri = (c >= p).astype(np.float32)
    # mzt: second slot of a diagonal pair: zeros below-band, then tri
    mzt = np.concatenate([np.zeros((P, P), np.float32), tri], axis=1)
    return tri, mzt


def _shard_inputs(x, w_qkv, b_qkv, w_proj, b_proj):
    tri, mzt = _make_masks()
    x = np.asarray(x, np.float32)
    w_qkv = np.asarray(w_qkv, np.float32)
    b_qkv = np.asarray(b_qkv, np.float32)
    w_proj = np.asarray(w_proj, np.float32)
    b_proj = np.asarray(b_proj, np.float32)
    zeros_c = np.zeros((C,), np.float32)
    in_maps = []
    for core in range(8):
        b, hg = core // 2, core % 2
        cs = slice(hg * HD, (hg + 1) * HD)
        wq = w_qkv[:, 0:C][:, cs]
        wk = w_qkv[:, C:2 * C][:, cs]
        wv = w_qkv[:, 2 * C:3 * C][:, cs]
        bq = b_qkv[0:C][cs]
        bk = b_qkv[C:2 * C][cs]
        bvv = b_qkv[2 * C:3 * C][cs]
        in_maps.append({
            "xT": np.ascontiguousarray(x[b].T).astype(BF16),
            "wqkv": np.ascontiguousarray(
                np.concatenate([wq, wk, wv], axis=1)).astype(BF16),
            "bqk": np.ascontiguousarray(np.concatenate([bq, bk])),
            "bv": np.ascontiguousarray(bvv),
            "wproj": np.ascontiguousarray(w_proj[cs, :]),
            "bproj": b_proj if hg == 0 else zeros_c,
            "mtrib": tri.astype(BF16),
            "mtri8": tri.astype(F8),
            "mzt8": mzt.astype(F8),
            "vones8": np.ones((P, HG), np.float32).astype(F8),
            "vonesb": np.ones((P, HG), np.float32).astype(BF16),
        })
    return in_maps


def get_program():
    global _RUNNER
    if _RUNNER is None:
        _RUNNER = _build_program()
    return _RUNNER


def kernel(x, w_qkv, b_qkv, w_proj, b_proj):
    nc = get_program()
    in_maps = _shard_inputs(x, w_qkv, b_qkv, w_proj, b_proj)
    res = run_bass_kernel_spmd(nc, in_maps, list(range(8)))
    out = np.empty((B, S, C), np.float32)
    for b in range(B):
        out[b] = res.results[2 * b]["out_part"] + res.results[2 * b + 1]["out_part"]
    return out
